# revision 9
# baseline (speedup 1.0000x reference)
"""Bass/Trainium2 kernel for BiDirectionalCrossAttention (8-core SPMD).

Sharding: 8 cores = 4 batches x 2 head-groups (4 heads each).

V2.1 design (vs baseline):
  - The steady loop is paced by ScalarE exp (64 x ~1.11us); all PE work
    (scores, attn@V, projections) is scheduled to hide under it.
  - Projections hoisted: qproj(.,0)/kproj(0,0)/vproj 0-3 run pre-loop so
    the exp stream starts ~5us in; remaining projections ride the loop
    slack in 256-wide halves.
  - attn PSUMs ([65,512] per head) are evicted to SBUF promptly at unit
    end (norm_start) so the next unit's accumulation never stalls.
  - reciprocal input packed [97,256] (4 x 256 q-halves on partitions
    0/32/64/96) - DVE reciprocal time scales with free size: 2x faster
    than [33,512].
  - out-proj split per (mo, t2): 2 matmuls + copy + DMA per group; only
    the 4 t2=1 groups (minus 2 pre-run m0 chunks) remain in the tail.
  - V ones-columns pre-memset in SBUF; vproj matmuls are plain 256-wide
    (no ones matmul, no interleaved weight upload).
PSUM: scores 1x[128,2,2,512] (4 banks) + attn 2 + proj pool 2 = 8.
Host folds V-bias through softmax: bout' = bout + Wout @ bv.
"""

import sys
import os

for _p in ("/opt/trn_rl_repo", "/root/.axon_site/_ro/trn_rl_repo"):
    if os.path.isdir(_p) and _p not in sys.path:
        sys.path.append(_p)

import numpy as np
import ml_dtypes

import concourse.bass as bass
import concourse.mybir as mybir
import concourse.tile as tile
from concourse.bass_utils import run_bass_kernel_spmd

BF16 = mybir.dt.bfloat16
F32 = mybir.dt.float32
NP_BF16 = ml_dtypes.bfloat16

AF = mybir.ActivationFunctionType


def _split_multi_waits(nc: bass.Bass) -> None:
    """The walrus build here allows only one sync-wait per instruction.
    Tile attaches several; hoist the extras onto same-engine NOPs placed
    immediately before the instruction (same per-engine program order)."""
    uid = 0
    for f in nc.m.functions:
        for bb in f.blocks:
            insts = bb.instructions
            out = []
            changed = False
            for inst in insts:
                si = inst.sync_info
                if si is not None and si.on_wait is not None and len(si.on_wait) > 1:
                    waits = list(si.on_wait)
                    for w in waits[:-1]:
                        nop = mybir.InstNoOp(
                            name=f"splitwait-{uid}",
                            engine=inst.engine,
                            ins=[],
                            outs=[],
                            sync_info=mybir.SyncInfo(on_wait=[w], on_update=[]),
                        )
                        uid += 1
                        out.append(nop)
                    inst.sync_info = mybir.SyncInfo(
                        on_wait=[waits[-1]], on_update=list(si.on_update or [])
                    )
                    changed = True
                out.append(inst)
            if changed:
                bb.instructions = out


def _build_program() -> bass.Bass:
    nc = bass.Bass()

    qx_d = nc.declare_dram_parameter("qx", [512, 1024], BF16, isOutput=False)
    kvx_d = nc.declare_dram_parameter("kvx", [512, 2048], BF16, isOutput=False)
    wq_d = nc.declare_dram_parameter("wq", [512, 256], BF16, isOutput=False)
    wk_d = nc.declare_dram_parameter("wk", [512, 256], BF16, isOutput=False)
    wv_d = nc.declare_dram_parameter("wv", [512, 256], BF16, isOutput=False)
    wo_d = nc.declare_dram_parameter("wo", [256, 512], BF16, isOutput=False)
    bq_d = nc.declare_dram_parameter("bq", [128, 2], F32, isOutput=False)
    bk_d = nc.declare_dram_parameter("bk", [128, 2], F32, isOutput=False)
    out_d = nc.declare_dram_parameter("out", [512, 1024], F32, isOutput=True)

    from contextlib import ExitStack

    with tile.TileContext(nc) as tc, ExitStack() as ctx:
        sb = ctx.enter_context(tc.tile_pool(name="sb", bufs=1))
        esb = ctx.enter_context(tc.tile_pool(name="esb", bufs=3))
        small = ctx.enter_context(tc.tile_pool(name="small", bufs=2))
        sc_ps = ctx.enter_context(tc.tile_pool(name="scps", bufs=1, space="PSUM"))
        at_ps = ctx.enter_context(tc.tile_pool(name="atps", bufs=1, space="PSUM"))
        pj_ps = ctx.enter_context(tc.tile_pool(name="pjps", bufs=2, space="PSUM"))
        dpool = ctx.enter_context(tc.tile_pool(name="dram", bufs=2, space="DRAM"))

        # ---------------- SBUF tiles (persistent) ----------------
        qx_s = sb.tile([128, 4, 1024], BF16, name="qx", tag="qx")
        kvx_s = sb.tile([128, 4, 2048], BF16, name="kvx", tag="kvx")
        wq_s = sb.tile([128, 4, 256], BF16, name="wq", tag="wq")
        wk_s = sb.tile([128, 4, 256], BF16, name="wk", tag="wk")
        wv_s = sb.tile([128, 4, 256], BF16, name="wv", tag="wv")
        wo_s = sb.tile([128, 2, 512], BF16, name="wo", tag="wo")
        bq_s = sb.tile([128, 2], F32, name="bq", tag="bq")
        bk_s = sb.tile([128, 2], F32, name="bk", tag="bk")
        qt_s = [sb.tile([128, 1024], BF16, name=f"qt{m}", tag=f"qt{m}") for m in range(2)]
        kt_s = [sb.tile([128, 2048], BF16, name=f"kt{m}", tag=f"kt{m}") for m in range(2)]
        v_s = sb.tile([128, 16, 260], BF16, name="v", tag="v")
        ot_s = [sb.tile([128, 1024], BF16, name=f"ot{m}", tag=f"ot{m}") for m in range(2)]

        # scores: one persistent 4-bank PSUM tile; [:, g%2, head, :] each a
        # full bank (no PSUM bank sharing between accumulation groups)
        sc_t = sc_ps.tile([128, 2, 2, 512], F32, name="sc", tag="sc")

        # v ones columns (col 65j+64 of each kv-tile row block) = 1.0
        for j in range(4):
            nc.vector.memset(v_s[:, :, 65 * j + 64:65 * j + 65], 1.0)

        # ---------------- DMAs in consumption order ----------------
        def chunked(d, parts=128):
            return d.rearrange("(k p) n -> p k n", p=parts)

        nc.sync.dma_start(out=wq_s[:], in_=chunked(wq_d))
        nc.sync.dma_start(out=bq_s[:], in_=bq_d[:])
        nc.sync.dma_start(out=qx_s[:, :, 0:512], in_=chunked(qx_d[:, 0:512]))
        nc.sync.dma_start(out=wk_s[:], in_=chunked(wk_d))
        nc.sync.dma_start(out=bk_s[:], in_=bk_d[:])
        nc.sync.dma_start(out=kvx_s[:, :, 0:512], in_=chunked(kvx_d[:, 0:512]))
        nc.sync.dma_start(out=wv_s[:], in_=chunked(wv_d))
        for t in range(1, 4):
            nc.sync.dma_start(out=kvx_s[:, :, t * 512:(t + 1) * 512],
                              in_=chunked(kvx_d[:, t * 512:(t + 1) * 512]))
        nc.sync.dma_start(out=qx_s[:, :, 512:1024], in_=chunked(qx_d[:, 512:1024]))
        nc.sync.dma_start(out=wo_s[:], in_=wo_d.rearrange("(m p) n -> p m n", p=128))

        # ---------------- building blocks ----------------
        def qproj_group(m, t):
            ps = pj_ps.tile([128, 512], F32, name="pp", tag="pp")
            for k in range(4):
                nc.tensor.matmul(
                    ps,
                    lhsT=wq_s[:, k, m * 128:(m + 1) * 128],
                    rhs=qx_s[:, k, t * 512:(t + 1) * 512],
                    start=(k == 0), stop=(k == 3),
                )
            nc.vector.tensor_scalar_add(
                out=qt_s[m][:, t * 512:(t + 1) * 512], in0=ps,
                scalar1=bq_s[:, m:m + 1],
            )

        def qproj_half(m, t, h):
            base = t * 512 + h * 256
            ps = pj_ps.tile([128, 256], F32, name="pp", tag="pp")
            for k in range(4):
                nc.tensor.matmul(
                    ps,
                    lhsT=wq_s[:, k, m * 128:(m + 1) * 128],
                    rhs=qx_s[:, k, base:base + 256],
                    start=(k == 0), stop=(k == 3),
                )
            nc.vector.tensor_scalar_add(
                out=qt_s[m][:, base:base + 256], in0=ps,
                scalar1=bq_s[:, m:m + 1],
            )

        def kproj_half(m, u, h):
            base = u * 512 + h * 256
            ps = pj_ps.tile([128, 256], F32, name="pp", tag="pp")
            for k in range(4):
                nc.tensor.matmul(
                    ps,
                    lhsT=wk_s[:, k, m * 128:(m + 1) * 128],
                    rhs=kvx_s[:, k, base:base + 256],
                    start=(k == 0), stop=(k == 3),
                )
            nc.vector.tensor_scalar_add(
                out=kt_s[m][:, base:base + 256], in0=ps,
                scalar1=bk_s[:, m:m + 1],
            )

        def vproj_tile(tt):
            # v_s[:, tt, 65j:65j+64] = token-major V for head j
            ps = pj_ps.tile([128, 256], F32, name="pp", tag="pp")
            for k in range(4):
                nc.tensor.matmul(
                    ps,
                    lhsT=kvx_s[:, k, tt * 128:(tt + 1) * 128],
                    rhs=wv_s[:, k, :],
                    start=(k == 0), stop=(k == 3),
                )
            src = ps.rearrange("p (j d) -> p j d", j=4)
            dst_row = v_s[:, tt, :]
            dst = bass.AP(
                tensor=dst_row.tensor, offset=dst_row.offset,
                ap=[list(dst_row.ap[0]), [65, 4], [1, 64]],
            )
            nc.vector.tensor_copy(out=dst, in_=src)

        # ---------------- attention ----------------
        # unit u: m = u%2 (head pair), t = u//2 (512-token half);
        # order (0,0),(1,0),(0,1),(1,1)
        def unit_mt(u):
            return u % 2, u // 2

        def emit_scores(g):
            u, i = g // 16, g % 16
            m, t = unit_mt(u)
            s = g % 2
            ksl = slice(i * 128, (i + 1) * 128)
            qsl = slice(t * 512, (t + 1) * 512)
            nc.tensor.matmul(
                sc_t[:, s, 0, :], lhsT=kt_s[m][0:64, ksl], rhs=qt_s[m][0:64, qsl],
                start=True, stop=True, tile_position=(0, 0),
            )
            nc.tensor.matmul(
                sc_t[:, s, 1, :], lhsT=kt_s[m][64:128, ksl], rhs=qt_s[m][64:128, qsl],
                start=True, stop=True, tile_position=(64, 0),
            )

        oh_tiles = {}

        def attn_iter(g, e_t):
            u, i = g // 16, g % 16
            m, _ = unit_mt(u)
            if i == 0:
                ohA = at_ps.tile([65, 512], F32, name="ohA", tag="ohA")
                ohB = at_ps.tile([65, 512], F32, name="ohB", tag="ohB")
                oh_tiles[u] = (ohA, ohB)
            ohA, ohB = oh_tiles[u]
            jA, jB = 2 * m, 2 * m + 1
            nc.tensor.matmul(
                ohA, lhsT=v_s[:, i, 65 * jA:65 * jA + 65], rhs=e_t[:, 0, :],
                start=(i == 0), stop=(i == 15),
            )
            nc.tensor.matmul(
                ohB, lhsT=v_s[:, i, 65 * jB:65 * jB + 65], rhs=e_t[:, 1, :],
                start=(i == 0), stop=(i == 15),
            )

        norm_state = {}

        def norm_start(u):
            # evict attn PSUMs to SBUF promptly (frees the 2 banks for the
            # next unit), pack denominators [97,256], reciprocal, stage to
            # DRAM for the partition-broadcast re-read
            ohA, ohB = oh_tiles.pop(u)
            ocp = small.tile([65, 2, 512], F32, name="ocp", tag="ocp", bufs=2)
            nc.vector.tensor_copy(out=ocp[:, 0, :], in_=ohA)
            nc.vector.tensor_copy(out=ocp[:, 1, :], in_=ohB)
            dst = small.tile([97, 256], F32, name="dst", tag="dst", bufs=2)
            for h in range(2):
                for q2 in range(2):
                    nc.vector.tensor_copy(
                        out=dst[64 * h + 32 * q2:64 * h + 32 * q2 + 1, :],
                        in_=ocp[64:65, h, 256 * q2:256 * q2 + 256],
                    )
            rec = small.tile([97, 256], F32, name="rec", tag="rec", bufs=2)
            nc.vector.reciprocal(out=rec, in_=dst)
            sd = dpool.tile([4, 256], F32, name="sd", tag="sd")
            for r in range(4):
                nc.sync.dma_start(out=sd[r:r + 1, :], in_=rec[32 * r:32 * r + 1, :])
            norm_state[u] = (ocp, sd)

        def norm_finish(u):
            m, t = unit_mt(u)
            qsl = slice(t * 512, (t + 1) * 512)
            ocp, sd = norm_state.pop(u)
            for h, base in ((0, 0), (1, 64)):
                row_ap = sd[2 * h:2 * h + 1, :]
                bsrc = bass.AP(tensor=row_ap.tensor, offset=row_ap.offset,
                               ap=[[0, 64], [1, 512]])
                bcs = small.tile([64, 512], F32, name="bcs", tag="bcs", bufs=4)
                nc.sync.dma_start(out=bcs, in_=bsrc)
                nc.vector.tensor_mul(ot_s[m][base:base + 64, qsl],
                                     ocp[0:64, h, :], bcs)

        # out-proj group (mo, t2): [128,512] psum, 2 matmuls (m=0,1)
        op_ps = {}

        def outproj_chunk(mo, t2, m, engine="vector"):
            key = (mo, t2)
            if key not in op_ps:
                op_ps[key] = pj_ps.tile([128, 512], F32, name="pp", tag="pp")
            ps = op_ps[key]
            nc.tensor.matmul(
                ps,
                lhsT=wo_s[:, m, mo * 128:(mo + 1) * 128],
                rhs=ot_s[m][:, t2 * 512:(t2 + 1) * 512],
                start=(m == 0), stop=(m == 1),
            )
            if m == 1:
                fo = small.tile([128, 512], F32, name="fo", tag="fo", bufs=2)
                if engine == "vector":
                    nc.vector.tensor_copy(out=fo, in_=ps)
                else:
                    nc.scalar.activation(out=fo, in_=ps, func=AF.Copy)
                nc.sync.dma_start(
                    out=out_d[mo * 128:(mo + 1) * 128, t2 * 512:(t2 + 1) * 512],
                    in_=fo,
                )
                del op_ps[key]

        def outproj_group(mo, t2, engine="vector"):
            outproj_chunk(mo, t2, 0, engine)
            outproj_chunk(mo, t2, 1, engine)

        # ---------------- pre-loop ----------------
        qproj_group(0, 0)
        kproj_half(0, 0, 0)
        kproj_half(0, 0, 1)
        emit_scores(0)
        qproj_group(1, 0)
        for tt in range(4):
            vproj_tile(tt)

        # ---------------- extras schedule (per iteration g) ----------------
        extras = {g: [] for g in range(64)}
        pre = {}
        # vproj tiles 4..15 at iters 0..11 (consumed at iter tt)
        for tt in range(4, 16):
            extras[tt - 4].append(lambda tt=tt: vproj_tile(tt))
        # kproj halves
        kplan = [(0, 1), (0, 2), (0, 3), (1, 0), (1, 1), (1, 2), (1, 3)]
        g = 0
        for (m_, u_) in kplan:
            for h_ in range(2):
                extras[g].append(lambda m_=m_, u_=u_, h_=h_: kproj_half(m_, u_, h_))
                g += 1
        # qproj halves for t=1 at g14..17
        g = 14
        for m_ in range(2):
            for h_ in range(2):
                extras[g].append(lambda m_=m_, h_=h_: qproj_half(m_, 1, h_))
                g += 1
        # norms: unit u ends at g=16u+15
        for u in range(3):
            pre[16 * u + 16] = [lambda u=u: norm_start(u)]
            extras[16 * u + 17].append(lambda u=u: norm_finish(u))
        # outproj t2=0 (units 0,1 normed by ~g34)
        for idx, mo in enumerate(range(4)):
            extras[36 + 4 * idx].append(lambda mo=mo: outproj_group(mo, 0))
        # outproj t2=1 m0 pre-run (unit 2 normed by ~g50)
        for mo in (0, 1):
            extras[52 + mo].append(lambda mo=mo: outproj_chunk(mo, 1, 0))

        # ---------------- main loop ----------------
        # Order matters for the counter-based semaphores: scores(g+1) must
        # precede attnV(g) in PE program order, else exp(g+1)'s wait
        # threshold transitively includes attnV(g) (which itself waits
        # exp(g)) and the two engines fully serialize.
        for g in range(64):
            if g + 1 < 64:
                emit_scores(g + 1)
            for fn in pre.get(g, ()):
                fn()
            e_t = esb.tile([128, 2, 512], BF16, name="e", tag="e")
            nc.scalar.activation(out=e_t[:], in_=sc_t[:, g % 2, :, :],
                                 func=AF.Exp, scale=0.125)
            for fn in extras.get(g, ()):
                fn()
            attn_iter(g, e_t)

        # ---------------- tail ----------------
        norm_start(3)
        norm_finish(3)
        for mo in (0, 1):
            outproj_chunk(mo, 1, 1, engine="scalar")
        outproj_group(2, 1, engine="scalar")
        outproj_group(3, 1, engine="vector")

    _split_multi_waits(nc)
    return nc


_PROGRAM = None


def _get_program() -> bass.Bass:
    global _PROGRAM
    if _PROGRAM is None:
        _PROGRAM = _build_program()
    return _PROGRAM


def _prep_core_inputs(c, q, kv, Wqkv, bqkv, Wout):
    b, g = c // 2, c % 2
    cs = slice(256 * g, 256 * g + 256)
    return {
        "qx": np.ascontiguousarray(q[b].reshape(512, 1024)).astype(NP_BF16),
        "kvx": np.ascontiguousarray(kv[b].reshape(512, 2048)).astype(NP_BF16),
        "wq": np.ascontiguousarray(Wqkv[cs, :].T).astype(NP_BF16),
        "wk": np.ascontiguousarray(Wqkv[512 + 256 * g:512 + 256 * g + 256, :].T).astype(NP_BF16),
        "wv": np.ascontiguousarray(Wqkv[1024 + 256 * g:1024 + 256 * g + 256, :].T).astype(NP_BF16),
        "wo": np.ascontiguousarray(Wout[:, cs].T).astype(NP_BF16),
        "bq": np.ascontiguousarray(bqkv[cs].reshape(2, 128).T).astype(np.float32),
        "bk": np.ascontiguousarray(bqkv[512 + 256 * g:512 + 256 * g + 256].reshape(2, 128).T).astype(np.float32),
    }


def kernel(q, kv, Wqkv, bqkv, Wout, bout):
    q = np.asarray(q, np.float32)
    kv = np.asarray(kv, np.float32)
    Wqkv = np.asarray(Wqkv, np.float32)
    bqkv = np.asarray(bqkv, np.float32)
    Wout = np.asarray(Wout, np.float32)
    bout = np.asarray(bout, np.float32)

    nc = _get_program()
    in_maps = [_prep_core_inputs(c, q, kv, Wqkv, bqkv, Wout) for c in range(8)]
    res = run_bass_kernel_spmd(nc, in_maps, list(range(8))).results

    # V-bias folds through softmax (rows sum to 1): bout' = bout + Wout @ bv
    bout_adj = bout + Wout @ bqkv[1024:1536]
    out = np.empty((4, 512, 32, 32), np.float32)
    for b in range(4):
        o = res[2 * b]["out"] + res[2 * b + 1]["out"] + bout_adj[:, None]
        out[b] = o.reshape(512, 32, 32)
    return out


# revision 12
# speedup vs baseline: 1.7160x; 1.7160x over previous
"""Bass/Trainium2 kernel for BiDirectionalCrossAttention (8-core SPMD).

Sharding: 8 cores = 4 batches x 2 head-groups (4 heads each).

V2.1 design (vs baseline):
  - The steady loop is paced by ScalarE exp (64 x ~1.11us); all PE work
    (scores, attn@V, projections) is scheduled to hide under it.
  - Projections hoisted: qproj(.,0)/kproj(0,0)/vproj 0-3 run pre-loop so
    the exp stream starts ~5us in; remaining projections ride the loop
    slack in 256-wide halves.
  - attn PSUMs ([65,512] per head) are evicted to SBUF promptly at unit
    end (norm_start) so the next unit's accumulation never stalls.
  - reciprocal input packed [97,256] (4 x 256 q-halves on partitions
    0/32/64/96) - DVE reciprocal time scales with free size: 2x faster
    than [33,512].
  - out-proj split per (mo, t2): 2 matmuls + copy + DMA per group; only
    the 4 t2=1 groups (minus 2 pre-run m0 chunks) remain in the tail.
  - V ones-columns pre-memset in SBUF; vproj matmuls are plain 256-wide
    (no ones matmul, no interleaved weight upload).
PSUM: scores 1x[128,2,2,512] (4 banks) + attn 2 + proj pool 2 = 8.
Host folds V-bias through softmax: bout' = bout + Wout @ bv.
"""

import sys
import os

for _p in ("/opt/trn_rl_repo", "/root/.axon_site/_ro/trn_rl_repo"):
    if os.path.isdir(_p) and _p not in sys.path:
        sys.path.append(_p)

import numpy as np
import ml_dtypes

import concourse.bass as bass
import concourse.mybir as mybir
import concourse.tile as tile
from concourse.bass_utils import run_bass_kernel_spmd

BF16 = mybir.dt.bfloat16
F32 = mybir.dt.float32
NP_BF16 = ml_dtypes.bfloat16

AF = mybir.ActivationFunctionType


def _split_multi_waits(nc: bass.Bass) -> None:
    """The walrus build here allows only one sync-wait per instruction.
    Tile attaches several; hoist the extras onto same-engine NOPs placed
    immediately before the instruction (same per-engine program order)."""
    uid = 0
    for f in nc.m.functions:
        for bb in f.blocks:
            insts = bb.instructions
            out = []
            changed = False
            for inst in insts:
                si = inst.sync_info
                if si is not None and si.on_wait is not None and len(si.on_wait) > 1:
                    waits = list(si.on_wait)
                    for w in waits[:-1]:
                        nop = mybir.InstNoOp(
                            name=f"splitwait-{uid}",
                            engine=inst.engine,
                            ins=[],
                            outs=[],
                            sync_info=mybir.SyncInfo(on_wait=[w], on_update=[]),
                        )
                        uid += 1
                        out.append(nop)
                    inst.sync_info = mybir.SyncInfo(
                        on_wait=[waits[-1]], on_update=list(si.on_update or [])
                    )
                    changed = True
                out.append(inst)
            if changed:
                bb.instructions = out


def _build_program() -> bass.Bass:
    nc = bass.Bass()

    qx_d = nc.declare_dram_parameter("qx", [512, 1024], BF16, isOutput=False)
    kvx_d = nc.declare_dram_parameter("kvx", [512, 2048], BF16, isOutput=False)
    wq_d = nc.declare_dram_parameter("wq", [512, 256], BF16, isOutput=False)
    wk_d = nc.declare_dram_parameter("wk", [512, 256], BF16, isOutput=False)
    wv_d = nc.declare_dram_parameter("wv", [512, 256], BF16, isOutput=False)
    wo_d = nc.declare_dram_parameter("wo", [256, 512], BF16, isOutput=False)
    bq_d = nc.declare_dram_parameter("bq", [128, 2], F32, isOutput=False)
    bk_d = nc.declare_dram_parameter("bk", [128, 2], F32, isOutput=False)
    out_d = nc.declare_dram_parameter("out", [512, 1024], F32, isOutput=True)

    from contextlib import ExitStack

    with tile.TileContext(nc) as tc, ExitStack() as ctx:
        sb = ctx.enter_context(tc.tile_pool(name="sb", bufs=1))
        esb = ctx.enter_context(tc.tile_pool(name="esb", bufs=3))
        small = ctx.enter_context(tc.tile_pool(name="small", bufs=2))
        sc_ps = ctx.enter_context(tc.tile_pool(name="scps", bufs=1, space="PSUM"))
        at_ps = ctx.enter_context(tc.tile_pool(name="atps", bufs=1, space="PSUM"))
        pj_ps = ctx.enter_context(tc.tile_pool(name="pjps", bufs=2, space="PSUM"))
        dpool = ctx.enter_context(tc.tile_pool(name="dram", bufs=2, space="DRAM"))

        # ---------------- SBUF tiles (persistent) ----------------
        qx_s = sb.tile([128, 4, 1024], BF16, name="qx", tag="qx")
        kvx_s = sb.tile([128, 4, 2048], BF16, name="kvx", tag="kvx")
        wq_s = sb.tile([128, 4, 256], BF16, name="wq", tag="wq")
        wk_s = sb.tile([128, 4, 256], BF16, name="wk", tag="wk")
        wv_s = sb.tile([128, 4, 256], BF16, name="wv", tag="wv")
        wo_s = sb.tile([128, 2, 512], BF16, name="wo", tag="wo")
        bq_s = sb.tile([128, 2], F32, name="bq", tag="bq")
        bk_s = sb.tile([128, 2], F32, name="bk", tag="bk")
        qt_s = [sb.tile([128, 1024], BF16, name=f"qt{m}", tag=f"qt{m}") for m in range(2)]
        kt_s = [sb.tile([128, 2048], BF16, name=f"kt{m}", tag=f"kt{m}") for m in range(2)]
        v_s = sb.tile([128, 16, 260], BF16, name="v", tag="v")
        ot_s = [sb.tile([128, 1024], BF16, name=f"ot{m}", tag=f"ot{m}") for m in range(2)]

        # scores: pool of [128,2,512] tiles (2 banks each, double-buffered).
        # Must be SEPARATE tile objects per iteration: a single persistent
        # tile with rotating slot slices makes exp(g) serialize against
        # scores(g+1) (coarse cross-slot dependency) and the loop runs 3x
        # slower.
        sc_tiles = {}

        # v ones columns (col 65j+64 of each kv-tile row block) = 1.0
        for j in range(4):
            nc.vector.memset(v_s[:, :, 65 * j + 64:65 * j + 65], 1.0)

        # ---------------- DMAs in consumption order ----------------
        def chunked(d, parts=128):
            return d.rearrange("(k p) n -> p k n", p=parts)

        nc.sync.dma_start(out=wq_s[:], in_=chunked(wq_d))
        nc.sync.dma_start(out=bq_s[:], in_=bq_d[:])
        nc.sync.dma_start(out=qx_s[:, :, 0:512], in_=chunked(qx_d[:, 0:512]))
        nc.sync.dma_start(out=wk_s[:], in_=chunked(wk_d))
        nc.sync.dma_start(out=bk_s[:], in_=bk_d[:])
        nc.sync.dma_start(out=kvx_s[:, :, 0:512], in_=chunked(kvx_d[:, 0:512]))
        nc.sync.dma_start(out=wv_s[:], in_=chunked(wv_d))
        for t in range(1, 4):
            nc.sync.dma_start(out=kvx_s[:, :, t * 512:(t + 1) * 512],
                              in_=chunked(kvx_d[:, t * 512:(t + 1) * 512]))
        nc.sync.dma_start(out=qx_s[:, :, 512:1024], in_=chunked(qx_d[:, 512:1024]))
        nc.sync.dma_start(out=wo_s[:], in_=wo_d.rearrange("(m p) n -> p m n", p=128))

        # ---------------- building blocks ----------------
        def qproj_group(m, t):
            ps = pj_ps.tile([128, 512], F32, name="pp", tag="pp")
            for k in range(4):
                nc.tensor.matmul(
                    ps,
                    lhsT=wq_s[:, k, m * 128:(m + 1) * 128],
                    rhs=qx_s[:, k, t * 512:(t + 1) * 512],
                    start=(k == 0), stop=(k == 3),
                )
            nc.vector.tensor_scalar_add(
                out=qt_s[m][:, t * 512:(t + 1) * 512], in0=ps,
                scalar1=bq_s[:, m:m + 1],
            )

        def qproj_half(m, t, h):
            base = t * 512 + h * 256
            ps = pj_ps.tile([128, 256], F32, name="pp", tag="pp")
            for k in range(4):
                nc.tensor.matmul(
                    ps,
                    lhsT=wq_s[:, k, m * 128:(m + 1) * 128],
                    rhs=qx_s[:, k, base:base + 256],
                    start=(k == 0), stop=(k == 3),
                )
            nc.vector.tensor_scalar_add(
                out=qt_s[m][:, base:base + 256], in0=ps,
                scalar1=bq_s[:, m:m + 1],
            )

        def kproj_half(m, u, h):
            base = u * 512 + h * 256
            ps = pj_ps.tile([128, 256], F32, name="pp", tag="pp")
            for k in range(4):
                nc.tensor.matmul(
                    ps,
                    lhsT=wk_s[:, k, m * 128:(m + 1) * 128],
                    rhs=kvx_s[:, k, base:base + 256],
                    start=(k == 0), stop=(k == 3),
                )
            nc.vector.tensor_scalar_add(
                out=kt_s[m][:, base:base + 256], in0=ps,
                scalar1=bk_s[:, m:m + 1],
            )

        def vproj_tile(tt):
            # v_s[:, tt, 65j:65j+64] = token-major V for head j
            ps = pj_ps.tile([128, 256], F32, name="pp", tag="pp")
            for k in range(4):
                nc.tensor.matmul(
                    ps,
                    lhsT=kvx_s[:, k, tt * 128:(tt + 1) * 128],
                    rhs=wv_s[:, k, :],
                    start=(k == 0), stop=(k == 3),
                )
            src = ps.rearrange("p (j d) -> p j d", j=4)
            dst_row = v_s[:, tt, :]
            dst = bass.AP(
                tensor=dst_row.tensor, offset=dst_row.offset,
                ap=[list(dst_row.ap[0]), [65, 4], [1, 64]],
            )
            nc.vector.tensor_copy(out=dst, in_=src)

        # ---------------- attention ----------------
        # unit u: m = u%2 (head pair), t = u//2 (512-token half);
        # order (0,0),(1,0),(0,1),(1,1)
        def unit_mt(u):
            return u % 2, u // 2

        def emit_scores(g):
            u, i = g // 16, g % 16
            m, t = unit_mt(u)
            ksl = slice(i * 128, (i + 1) * 128)
            qsl = slice(t * 512, (t + 1) * 512)
            sc = sc_ps.tile([128, 2, 512], F32, name="sc", tag="sc", bufs=2)
            nc.tensor.matmul(
                sc[:, 0, :], lhsT=kt_s[m][0:64, ksl], rhs=qt_s[m][0:64, qsl],
                start=True, stop=True, tile_position=(0, 0),
            )
            nc.tensor.matmul(
                sc[:, 1, :], lhsT=kt_s[m][64:128, ksl], rhs=qt_s[m][64:128, qsl],
                start=True, stop=True, tile_position=(64, 0),
            )
            sc_tiles[g] = sc

        oh_tiles = {}

        def attn_iter(g, e_t):
            u, i = g // 16, g % 16
            m, _ = unit_mt(u)
            if i == 0:
                ohA = at_ps.tile([65, 512], F32, name="ohA", tag="ohA")
                ohB = at_ps.tile([65, 512], F32, name="ohB", tag="ohB")
                oh_tiles[u] = (ohA, ohB)
            ohA, ohB = oh_tiles[u]
            jA, jB = 2 * m, 2 * m + 1
            nc.tensor.matmul(
                ohA, lhsT=v_s[:, i, 65 * jA:65 * jA + 65], rhs=e_t[:, 0, :],
                start=(i == 0), stop=(i == 15),
            )
            nc.tensor.matmul(
                ohB, lhsT=v_s[:, i, 65 * jB:65 * jB + 65], rhs=e_t[:, 1, :],
                start=(i == 0), stop=(i == 15),
            )

        norm_state = {}

        def norm_start(u):
            # evict attn PSUMs to SBUF promptly (frees the 2 banks for the
            # next unit), pack denominators [97,256], reciprocal, stage to
            # DRAM for the partition-broadcast re-read
            ohA, ohB = oh_tiles.pop(u)
            ocp = small.tile([65, 2, 512], F32, name="ocp", tag="ocp", bufs=2)
            nc.vector.tensor_copy(out=ocp[:, 0, :], in_=ohA)
            nc.vector.tensor_copy(out=ocp[:, 1, :], in_=ohB)
            dst = small.tile([97, 256], F32, name="dst", tag="dst", bufs=2)
            for h in range(2):
                for q2 in range(2):
                    nc.vector.tensor_copy(
                        out=dst[64 * h + 32 * q2:64 * h + 32 * q2 + 1, :],
                        in_=ocp[64:65, h, 256 * q2:256 * q2 + 256],
                    )
            rec = small.tile([97, 256], F32, name="rec", tag="rec", bufs=2)
            nc.vector.reciprocal(out=rec, in_=dst)
            sd = dpool.tile([4, 256], F32, name="sd", tag="sd")
            for r in range(4):
                nc.sync.dma_start(out=sd[r:r + 1, :], in_=rec[32 * r:32 * r + 1, :])
            norm_state[u] = (ocp, sd)

        def norm_finish(u):
            m, t = unit_mt(u)
            qsl = slice(t * 512, (t + 1) * 512)
            ocp, sd = norm_state.pop(u)
            for h, base in ((0, 0), (1, 64)):
                row_ap = sd[2 * h:2 * h + 1, :]
                bsrc = bass.AP(tensor=row_ap.tensor, offset=row_ap.offset,
                               ap=[[0, 64], [1, 512]])
                bcs = small.tile([64, 512], F32, name="bcs", tag="bcs", bufs=4)
                nc.sync.dma_start(out=bcs, in_=bsrc)
                nc.vector.tensor_mul(ot_s[m][base:base + 64, qsl],
                                     ocp[0:64, h, :], bcs)

        # out-proj group (mo, t2): [128,512] psum, 2 matmuls (m=0,1)
        op_ps = {}

        def outproj_chunk(mo, t2, m, engine="vector"):
            key = (mo, t2)
            if key not in op_ps:
                op_ps[key] = pj_ps.tile([128, 512], F32, name="pp", tag="pp")
            ps = op_ps[key]
            nc.tensor.matmul(
                ps,
                lhsT=wo_s[:, m, mo * 128:(mo + 1) * 128],
                rhs=ot_s[m][:, t2 * 512:(t2 + 1) * 512],
                start=(m == 0), stop=(m == 1),
            )
            if m == 1:
                fo = small.tile([128, 512], F32, name="fo", tag="fo", bufs=2)
                if engine == "vector":
                    nc.vector.tensor_copy(out=fo, in_=ps)
                else:
                    nc.scalar.activation(out=fo, in_=ps, func=AF.Copy)
                nc.sync.dma_start(
                    out=out_d[mo * 128:(mo + 1) * 128, t2 * 512:(t2 + 1) * 512],
                    in_=fo,
                )
                del op_ps[key]

        def outproj_group(mo, t2, engine="vector"):
            outproj_chunk(mo, t2, 0, engine)
            outproj_chunk(mo, t2, 1, engine)

        # ---------------- pre-loop ----------------
        qproj_group(0, 0)
        kproj_half(0, 0, 0)
        kproj_half(0, 0, 1)
        emit_scores(0)
        qproj_group(1, 0)
        for tt in range(4):
            vproj_tile(tt)

        # ---------------- extras schedule (per iteration g) ----------------
        extras = {g: [] for g in range(64)}
        pre = {}
        # vproj tiles 4..15 at iters 0..11 (consumed at iter tt)
        for tt in range(4, 16):
            extras[tt - 4].append(lambda tt=tt: vproj_tile(tt))
        # kproj halves
        kplan = [(0, 1), (0, 2), (0, 3), (1, 0), (1, 1), (1, 2), (1, 3)]
        g = 0
        for (m_, u_) in kplan:
            for h_ in range(2):
                extras[g].append(lambda m_=m_, u_=u_, h_=h_: kproj_half(m_, u_, h_))
                g += 1
        # qproj halves for t=1 at g14..17
        g = 14
        for m_ in range(2):
            for h_ in range(2):
                extras[g].append(lambda m_=m_, h_=h_: qproj_half(m_, 1, h_))
                g += 1
        # norms: unit u ends at g=16u+15
        for u in range(3):
            pre[16 * u + 16] = [lambda u=u: norm_start(u)]
            extras[16 * u + 17].append(lambda u=u: norm_finish(u))
        # outproj t2=0 (units 0,1 normed by ~g34)
        for idx, mo in enumerate(range(4)):
            extras[36 + 4 * idx].append(lambda mo=mo: outproj_group(mo, 0))
        # outproj t2=1 m0 pre-run (unit 2 normed by ~g50)
        for mo in (0, 1):
            extras[52 + mo].append(lambda mo=mo: outproj_chunk(mo, 1, 0))

        # ---------------- main loop ----------------
        # Order matters for the counter-based semaphores: scores(g+1) must
        # precede attnV(g) in PE program order, else exp(g+1)'s wait
        # threshold transitively includes attnV(g) (which itself waits
        # exp(g)) and the two engines fully serialize.
        for g in range(64):
            if g + 1 < 64:
                emit_scores(g + 1)
            for fn in pre.get(g, ()):
                fn()
            sc = sc_tiles.pop(g)
            e_t = esb.tile([128, 2, 512], BF16, name="e", tag="e")
            nc.scalar.activation(out=e_t[:], in_=sc[:],
                                 func=AF.Exp, scale=0.125)
            for fn in extras.get(g, ()):
                fn()
            attn_iter(g, e_t)

        # ---------------- tail ----------------
        norm_start(3)
        norm_finish(3)
        for mo in (0, 1):
            outproj_chunk(mo, 1, 1, engine="scalar")
        outproj_group(2, 1, engine="scalar")
        outproj_group(3, 1, engine="vector")

    _split_multi_waits(nc)
    return nc


_PROGRAM = None


def _get_program() -> bass.Bass:
    global _PROGRAM
    if _PROGRAM is None:
        _PROGRAM = _build_program()
    return _PROGRAM


def _prep_core_inputs(c, q, kv, Wqkv, bqkv, Wout):
    b, g = c // 2, c % 2
    cs = slice(256 * g, 256 * g + 256)
    return {
        "qx": np.ascontiguousarray(q[b].reshape(512, 1024)).astype(NP_BF16),
        "kvx": np.ascontiguousarray(kv[b].reshape(512, 2048)).astype(NP_BF16),
        "wq": np.ascontiguousarray(Wqkv[cs, :].T).astype(NP_BF16),
        "wk": np.ascontiguousarray(Wqkv[512 + 256 * g:512 + 256 * g + 256, :].T).astype(NP_BF16),
        "wv": np.ascontiguousarray(Wqkv[1024 + 256 * g:1024 + 256 * g + 256, :].T).astype(NP_BF16),
        "wo": np.ascontiguousarray(Wout[:, cs].T).astype(NP_BF16),
        "bq": np.ascontiguousarray(bqkv[cs].reshape(2, 128).T).astype(np.float32),
        "bk": np.ascontiguousarray(bqkv[512 + 256 * g:512 + 256 * g + 256].reshape(2, 128).T).astype(np.float32),
    }


def kernel(q, kv, Wqkv, bqkv, Wout, bout):
    q = np.asarray(q, np.float32)
    kv = np.asarray(kv, np.float32)
    Wqkv = np.asarray(Wqkv, np.float32)
    bqkv = np.asarray(bqkv, np.float32)
    Wout = np.asarray(Wout, np.float32)
    bout = np.asarray(bout, np.float32)

    nc = _get_program()
    in_maps = [_prep_core_inputs(c, q, kv, Wqkv, bqkv, Wout) for c in range(8)]
    res = run_bass_kernel_spmd(nc, in_maps, list(range(8))).results

    # V-bias folds through softmax (rows sum to 1): bout' = bout + Wout @ bv
    bout_adj = bout + Wout @ bqkv[1024:1536]
    out = np.empty((4, 512, 32, 32), np.float32)
    for b in range(4):
        o = res[2 * b]["out"] + res[2 * b + 1]["out"] + bout_adj[:, None]
        out[b] = o.reshape(512, 32, 32)
    return out


# revision 18
# speedup vs baseline: 1.7204x; 1.0026x over previous
"""Bass/Trainium2 kernel for BiDirectionalCrossAttention (8-core SPMD).

Sharding: 8 cores = 4 batches x 2 head-groups (4 heads each).

V2.1 design (vs baseline):
  - The steady loop is paced by ScalarE exp (64 x ~1.11us); all PE work
    (scores, attn@V, projections) is scheduled to hide under it.
  - Projections hoisted: qproj(.,0)/kproj(0,0)/vproj 0-3 run pre-loop so
    the exp stream starts ~5us in; remaining projections ride the loop
    slack in 256-wide halves.
  - attn PSUMs ([65,512] per head) are evicted to SBUF promptly at unit
    end (norm_start) so the next unit's accumulation never stalls.
  - reciprocal input packed [97,256] (4 x 256 q-halves on partitions
    0/32/64/96) - DVE reciprocal time scales with free size: 2x faster
    than [33,512].
  - out-proj split per (mo, t2): 2 matmuls + copy + DMA per group; only
    the 4 t2=1 groups (minus 2 pre-run m0 chunks) remain in the tail.
  - V ones-columns pre-memset in SBUF; vproj matmuls are plain 256-wide
    (no ones matmul, no interleaved weight upload).
PSUM: scores 1x[128,2,2,512] (4 banks) + attn 2 + proj pool 2 = 8.
Host folds V-bias through softmax: bout' = bout + Wout @ bv.
"""

import sys
import os

for _p in ("/opt/trn_rl_repo", "/root/.axon_site/_ro/trn_rl_repo"):
    if os.path.isdir(_p) and _p not in sys.path:
        sys.path.append(_p)

import numpy as np
import ml_dtypes

import concourse.bass as bass
import concourse.mybir as mybir
import concourse.tile as tile
from concourse.bass_utils import run_bass_kernel_spmd

BF16 = mybir.dt.bfloat16
F32 = mybir.dt.float32
NP_BF16 = ml_dtypes.bfloat16

AF = mybir.ActivationFunctionType


def _split_multi_waits(nc: bass.Bass) -> None:
    """The walrus build here allows only one sync-wait per instruction.
    Tile attaches several; hoist the extras onto same-engine NOPs placed
    immediately before the instruction (same per-engine program order)."""
    uid = 0
    for f in nc.m.functions:
        for bb in f.blocks:
            insts = bb.instructions
            out = []
            changed = False
            for inst in insts:
                si = inst.sync_info
                if si is not None and si.on_wait is not None and len(si.on_wait) > 1:
                    waits = list(si.on_wait)
                    for w in waits[:-1]:
                        nop = mybir.InstNoOp(
                            name=f"splitwait-{uid}",
                            engine=inst.engine,
                            ins=[],
                            outs=[],
                            sync_info=mybir.SyncInfo(on_wait=[w], on_update=[]),
                        )
                        uid += 1
                        out.append(nop)
                    inst.sync_info = mybir.SyncInfo(
                        on_wait=[waits[-1]], on_update=list(si.on_update or [])
                    )
                    changed = True
                out.append(inst)
            if changed:
                bb.instructions = out


def _build_program() -> bass.Bass:
    nc = bass.Bass()

    qx_d = nc.declare_dram_parameter("qx", [512, 1024], BF16, isOutput=False)
    kvx_d = nc.declare_dram_parameter("kvx", [512, 2048], BF16, isOutput=False)
    wq_d = nc.declare_dram_parameter("wq", [512, 256], BF16, isOutput=False)
    wk_d = nc.declare_dram_parameter("wk", [512, 256], BF16, isOutput=False)
    wv_d = nc.declare_dram_parameter("wv", [512, 256], BF16, isOutput=False)
    wo_d = nc.declare_dram_parameter("wo", [256, 512], BF16, isOutput=False)
    bq_d = nc.declare_dram_parameter("bq", [128, 2], F32, isOutput=False)
    bk_d = nc.declare_dram_parameter("bk", [128, 2], F32, isOutput=False)
    out_d = nc.declare_dram_parameter("out", [512, 1024], F32, isOutput=True)

    from contextlib import ExitStack

    with tile.TileContext(nc) as tc, ExitStack() as ctx:
        sb = ctx.enter_context(tc.tile_pool(name="sb", bufs=1))
        esb = ctx.enter_context(tc.tile_pool(name="esb", bufs=4))
        small = ctx.enter_context(tc.tile_pool(name="small", bufs=2))
        sc_ps = ctx.enter_context(tc.tile_pool(name="scps", bufs=1, space="PSUM"))
        at_ps = ctx.enter_context(tc.tile_pool(name="atps", bufs=1, space="PSUM"))
        pj_ps = ctx.enter_context(tc.tile_pool(name="pjps", bufs=2, space="PSUM"))
        dpool = ctx.enter_context(tc.tile_pool(name="dram", bufs=2, space="DRAM"))

        # ---------------- SBUF tiles (persistent) ----------------
        qx_s = sb.tile([128, 4, 1024], BF16, name="qx", tag="qx")
        kvx_s = sb.tile([128, 4, 2048], BF16, name="kvx", tag="kvx")
        wq_s = sb.tile([128, 4, 256], BF16, name="wq", tag="wq")
        wk_s = sb.tile([128, 4, 256], BF16, name="wk", tag="wk")
        wv_s = sb.tile([128, 4, 256], BF16, name="wv", tag="wv")
        wo_s = sb.tile([128, 2, 512], BF16, name="wo", tag="wo")
        bq_s = sb.tile([128, 2], F32, name="bq", tag="bq")
        bk_s = sb.tile([128, 2], F32, name="bk", tag="bk")
        qt_s = [sb.tile([128, 1024], BF16, name=f"qt{m}", tag=f"qt{m}") for m in range(2)]
        kt_s = [sb.tile([128, 2048], BF16, name=f"kt{m}", tag=f"kt{m}") for m in range(2)]
        v_s = sb.tile([128, 16, 260], BF16, name="v", tag="v")
        ot_s = [sb.tile([128, 1024], BF16, name=f"ot{m}", tag=f"ot{m}") for m in range(2)]

        # scores: pool of [128,2,512] tiles (2 banks each, double-buffered).
        # Must be SEPARATE tile objects per iteration: a single persistent
        # tile with rotating slot slices makes exp(g) serialize against
        # scores(g+1) (coarse cross-slot dependency) and the loop runs 3x
        # slower.
        sc_tiles = {}

        # v ones columns (col 65j+64 of each kv-tile row block) = 1.0
        for j in range(4):
            nc.vector.memset(v_s[:, :, 65 * j + 64:65 * j + 65], 1.0)

        # ---------------- DMAs in consumption order ----------------
        def chunked(d, parts=128):
            return d.rearrange("(k p) n -> p k n", p=parts)

        # fine-grained first transfers so qproj/kproj/scores(0) start ASAP
        nc.sync.dma_start(out=wq_s[:], in_=chunked(wq_d))
        nc.sync.dma_start(out=qx_s[:, :, 0:256], in_=chunked(qx_d[:, 0:256]))
        nc.sync.dma_start(out=bq_s[:], in_=bq_d[:])
        nc.sync.dma_start(out=qx_s[:, :, 256:512], in_=chunked(qx_d[:, 256:512]))
        nc.sync.dma_start(out=wk_s[:], in_=chunked(wk_d))
        nc.sync.dma_start(out=bk_s[:], in_=bk_d[:])
        nc.sync.dma_start(out=kvx_s[:, :, 0:256], in_=chunked(kvx_d[:, 0:256]))
        nc.sync.dma_start(out=wv_s[:], in_=chunked(wv_d))
        nc.sync.dma_start(out=kvx_s[:, :, 256:512], in_=chunked(kvx_d[:, 256:512]))
        for t in range(1, 4):
            nc.sync.dma_start(out=kvx_s[:, :, t * 512:(t + 1) * 512],
                              in_=chunked(kvx_d[:, t * 512:(t + 1) * 512]))
        nc.sync.dma_start(out=qx_s[:, :, 512:1024], in_=chunked(qx_d[:, 512:1024]))
        nc.sync.dma_start(out=wo_s[:], in_=wo_d.rearrange("(m p) n -> p m n", p=128))

        # ---------------- building blocks ----------------
        def qproj_group(m, t):
            ps = pj_ps.tile([128, 512], F32, name="pp", tag="pp")
            for k in range(4):
                nc.tensor.matmul(
                    ps,
                    lhsT=wq_s[:, k, m * 128:(m + 1) * 128],
                    rhs=qx_s[:, k, t * 512:(t + 1) * 512],
                    start=(k == 0), stop=(k == 3),
                )
            nc.vector.tensor_scalar_add(
                out=qt_s[m][:, t * 512:(t + 1) * 512], in0=ps,
                scalar1=bq_s[:, m:m + 1],
            )

        def qproj_half(m, t, h):
            base = t * 512 + h * 256
            ps = pj_ps.tile([128, 256], F32, name="pp", tag="pp")
            for k in range(4):
                nc.tensor.matmul(
                    ps,
                    lhsT=wq_s[:, k, m * 128:(m + 1) * 128],
                    rhs=qx_s[:, k, base:base + 256],
                    start=(k == 0), stop=(k == 3),
                )
            nc.vector.tensor_scalar_add(
                out=qt_s[m][:, base:base + 256], in0=ps,
                scalar1=bq_s[:, m:m + 1],
            )

        def kproj_half(m, u, h):
            base = u * 512 + h * 256
            ps = pj_ps.tile([128, 256], F32, name="pp", tag="pp")
            for k in range(4):
                nc.tensor.matmul(
                    ps,
                    lhsT=wk_s[:, k, m * 128:(m + 1) * 128],
                    rhs=kvx_s[:, k, base:base + 256],
                    start=(k == 0), stop=(k == 3),
                )
            nc.vector.tensor_scalar_add(
                out=kt_s[m][:, base:base + 256], in0=ps,
                scalar1=bk_s[:, m:m + 1],
            )

        def vproj_tile(tt):
            # v_s[:, tt, 65j:65j+64] = token-major V for head j
            ps = pj_ps.tile([128, 256], F32, name="pp", tag="pp")
            for k in range(4):
                nc.tensor.matmul(
                    ps,
                    lhsT=kvx_s[:, k, tt * 128:(tt + 1) * 128],
                    rhs=wv_s[:, k, :],
                    start=(k == 0), stop=(k == 3),
                )
            src = ps.rearrange("p (j d) -> p j d", j=4)
            dst_row = v_s[:, tt, :]
            dst = bass.AP(
                tensor=dst_row.tensor, offset=dst_row.offset,
                ap=[list(dst_row.ap[0]), [65, 4], [1, 64]],
            )
            nc.vector.tensor_copy(out=dst, in_=src)

        # ---------------- attention ----------------
        # unit u: m = u%2 (head pair), t = u//2 (512-token half);
        # order (0,0),(1,0),(0,1),(1,1)
        def unit_mt(u):
            return u % 2, u // 2

        def emit_scores(g):
            u, i = g // 16, g % 16
            m, t = unit_mt(u)
            ksl = slice(i * 128, (i + 1) * 128)
            qsl = slice(t * 512, (t + 1) * 512)
            sc = sc_ps.tile([128, 2, 512], F32, name="sc", tag="sc", bufs=2)
            nc.tensor.matmul(
                sc[:, 0, :], lhsT=kt_s[m][0:64, ksl], rhs=qt_s[m][0:64, qsl],
                start=True, stop=True, tile_position=(0, 0),
            )
            nc.tensor.matmul(
                sc[:, 1, :], lhsT=kt_s[m][64:128, ksl], rhs=qt_s[m][64:128, qsl],
                start=True, stop=True, tile_position=(64, 0),
            )
            sc_tiles[g] = sc

        oh_tiles = {}

        def attn_iter(g, e_t):
            u, i = g // 16, g % 16
            m, _ = unit_mt(u)
            if i == 0:
                ohA = at_ps.tile([65, 512], F32, name="ohA", tag="ohA")
                ohB = at_ps.tile([65, 512], F32, name="ohB", tag="ohB")
                oh_tiles[u] = (ohA, ohB)
            ohA, ohB = oh_tiles[u]
            jA, jB = 2 * m, 2 * m + 1
            nc.tensor.matmul(
                ohA, lhsT=v_s[:, i, 65 * jA:65 * jA + 65], rhs=e_t[:, 0, :],
                start=(i == 0), stop=(i == 15),
            )
            nc.tensor.matmul(
                ohB, lhsT=v_s[:, i, 65 * jB:65 * jB + 65], rhs=e_t[:, 1, :],
                start=(i == 0), stop=(i == 15),
            )

        norm_state = {}

        def norm_start(u, evict=True):
            # evict attn PSUMs to SBUF promptly (frees the 2 banks for the
            # next unit), pack denominators [97,256], reciprocal, stage to
            # DRAM for the partition-broadcast re-read. The last unit skips
            # the eviction (nobody reuses its banks) and reads PSUM direct.
            ohA, ohB = oh_tiles.pop(u)
            if evict:
                ocp = small.tile([65, 2, 512], F32, name="ocp", tag="ocp", bufs=2)
                nc.vector.tensor_copy(out=ocp[:, 0, :], in_=ohA)
                nc.vector.tensor_copy(out=ocp[:, 1, :], in_=ohB)
                srcs = (ocp[:, 0, :], ocp[:, 1, :])
            else:
                srcs = (ohA, ohB)
            dst = small.tile([97, 256], F32, name="dst", tag="dst", bufs=2)
            for h in range(2):
                for q2 in range(2):
                    nc.vector.tensor_copy(
                        out=dst[64 * h + 32 * q2:64 * h + 32 * q2 + 1, :],
                        in_=srcs[h][64:65, 256 * q2:256 * q2 + 256],
                    )
            rec = small.tile([97, 256], F32, name="rec", tag="rec", bufs=2)
            nc.vector.reciprocal(out=rec, in_=dst)
            sd = dpool.tile([4, 256], F32, name="sd", tag="sd")
            for r in range(4):
                nc.sync.dma_start(out=sd[r:r + 1, :], in_=rec[32 * r:32 * r + 1, :])
            norm_state[u] = (srcs, sd)

        def norm_finish(u):
            m, t = unit_mt(u)
            qsl = slice(t * 512, (t + 1) * 512)
            srcs, sd = norm_state.pop(u)
            for h, base in ((0, 0), (1, 64)):
                row_ap = sd[2 * h:2 * h + 1, :]
                bsrc = bass.AP(tensor=row_ap.tensor, offset=row_ap.offset,
                               ap=[[0, 64], [1, 512]])
                bcs = small.tile([64, 512], F32, name="bcs", tag="bcs", bufs=4)
                nc.sync.dma_start(out=bcs, in_=bsrc)
                nc.vector.tensor_mul(ot_s[m][base:base + 64, qsl],
                                     srcs[h][0:64, :], bcs)

        # out-proj group (mo, t2): [128,512] psum, 2 matmuls (m=0,1)
        op_ps = {}

        def outproj_chunk(mo, t2, m, engine="vector"):
            key = (mo, t2)
            if key not in op_ps:
                op_ps[key] = pj_ps.tile([128, 512], F32, name="pp", tag="pp")
            ps = op_ps[key]
            nc.tensor.matmul(
                ps,
                lhsT=wo_s[:, m, mo * 128:(mo + 1) * 128],
                rhs=ot_s[m][:, t2 * 512:(t2 + 1) * 512],
                start=(m == 0), stop=(m == 1),
            )
            if m == 1:
                fo = small.tile([128, 512], F32, name="fo", tag="fo", bufs=2)
                if engine == "vector":
                    nc.vector.tensor_copy(out=fo, in_=ps)
                else:
                    nc.scalar.activation(out=fo, in_=ps, func=AF.Copy)
                nc.sync.dma_start(
                    out=out_d[mo * 128:(mo + 1) * 128, t2 * 512:(t2 + 1) * 512],
                    in_=fo,
                )
                del op_ps[key]

        def outproj_group(mo, t2, engine="vector"):
            outproj_chunk(mo, t2, 0, engine)
            outproj_chunk(mo, t2, 1, engine)

        # ---------------- pre-loop ----------------
        # minimal critical path to scores(0)/exp(0), then the rest
        qproj_half(0, 0, 0)
        qproj_half(0, 0, 1)
        kproj_half(0, 0, 0)
        kproj_half(0, 0, 1)
        emit_scores(0)
        qproj_half(1, 0, 0)
        qproj_half(1, 0, 1)
        for tt in range(4):
            vproj_tile(tt)

        # ---------------- extras schedule (per iteration g) ----------------
        extras = {g: [] for g in range(64)}
        pre = {}
        # kproj(0,1) at 0,1; (0,2) at 2,3; (0,3) at 6,7 (needed by 4/8/12)
        for idx, (u_, g0) in enumerate(((1, 0), (2, 2), (3, 6))):
            for h_ in range(2):
                extras[g0 + h_].append(lambda u_=u_, h_=h_: kproj_half(0, u_, h_))
        # vproj tiles 4..15 just-in-time-2 at iters 2..13
        for tt in range(4, 16):
            extras[max(0, tt - 2)].append(lambda tt=tt: vproj_tile(tt))
        # kproj(1,u) at 14..21 (needed by 16/20/24/28)
        g = 14
        for u_ in range(4):
            for h_ in range(2):
                extras[g].append(lambda u_=u_, h_=h_: kproj_half(1, u_, h_))
                g += 1
        # qproj halves for t=1 at g22..25 (needed by 32/48)
        g = 22
        for m_ in range(2):
            for h_ in range(2):
                extras[g].append(lambda m_=m_, h_=h_: qproj_half(m_, 1, h_))
                g += 1
        # norms: unit u ends at g=16u+15
        for u in range(3):
            pre[16 * u + 16] = [lambda u=u: norm_start(u)]
            extras[16 * u + 17].append(lambda u=u: norm_finish(u))
        # outproj t2=0 (units 0,1 normed by ~g35)
        for idx, mo in enumerate(range(4)):
            extras[38 + 4 * idx].append(lambda mo=mo: outproj_group(mo, 0))

        # ---------------- main loop ----------------
        # Order matters for the counter-based semaphores: scores(g+1) must
        # precede attnV(g) in PE program order, else exp(g+1)'s wait
        # threshold transitively includes attnV(g) (which itself waits
        # exp(g)) and the two engines fully serialize.
        for g in range(64):
            if g + 1 < 64:
                emit_scores(g + 1)
            for fn in pre.get(g, ()):
                fn()
            sc = sc_tiles.pop(g)
            e_t = esb.tile([128, 2, 512], BF16, name="e", tag="e")
            nc.scalar.activation(out=e_t[:], in_=sc[:],
                                 func=AF.Exp, scale=0.125)
            for fn in extras.get(g, ()):
                fn()
            attn_iter(g, e_t)

        # ---------------- tail ----------------
        norm_start(3, evict=False)
        norm_finish(3)
        outproj_group(0, 1, engine="scalar")
        outproj_group(1, 1, engine="vector")
        outproj_group(2, 1, engine="scalar")
        outproj_group(3, 1, engine="vector")

    _split_multi_waits(nc)
    return nc


_PROGRAM = None


def _get_program() -> bass.Bass:
    global _PROGRAM
    if _PROGRAM is None:
        _PROGRAM = _build_program()
    return _PROGRAM


def _prep_core_inputs(c, q, kv, Wqkv, bqkv, Wout):
    b, g = c // 2, c % 2
    cs = slice(256 * g, 256 * g + 256)
    return {
        "qx": np.ascontiguousarray(q[b].reshape(512, 1024)).astype(NP_BF16),
        "kvx": np.ascontiguousarray(kv[b].reshape(512, 2048)).astype(NP_BF16),
        "wq": np.ascontiguousarray(Wqkv[cs, :].T).astype(NP_BF16),
        "wk": np.ascontiguousarray(Wqkv[512 + 256 * g:512 + 256 * g + 256, :].T).astype(NP_BF16),
        "wv": np.ascontiguousarray(Wqkv[1024 + 256 * g:1024 + 256 * g + 256, :].T).astype(NP_BF16),
        "wo": np.ascontiguousarray(Wout[:, cs].T).astype(NP_BF16),
        "bq": np.ascontiguousarray(bqkv[cs].reshape(2, 128).T).astype(np.float32),
        "bk": np.ascontiguousarray(bqkv[512 + 256 * g:512 + 256 * g + 256].reshape(2, 128).T).astype(np.float32),
    }


def kernel(q, kv, Wqkv, bqkv, Wout, bout):
    q = np.asarray(q, np.float32)
    kv = np.asarray(kv, np.float32)
    Wqkv = np.asarray(Wqkv, np.float32)
    bqkv = np.asarray(bqkv, np.float32)
    Wout = np.asarray(Wout, np.float32)
    bout = np.asarray(bout, np.float32)

    nc = _get_program()
    in_maps = [_prep_core_inputs(c, q, kv, Wqkv, bqkv, Wout) for c in range(8)]
    res = run_bass_kernel_spmd(nc, in_maps, list(range(8))).results

    # V-bias folds through softmax (rows sum to 1): bout' = bout + Wout @ bv
    bout_adj = bout + Wout @ bqkv[1024:1536]
    out = np.empty((4, 512, 32, 32), np.float32)
    for b in range(4):
        o = res[2 * b]["out"] + res[2 * b + 1]["out"] + bout_adj[:, None]
        out[b] = o.reshape(512, 32, 32)
    return out


# revision 24
# speedup vs baseline: 1.8471x; 1.0736x over previous
"""Bass/Trainium2 kernel for BiDirectionalCrossAttention (8-core SPMD).

Sharding: 8 cores = 4 batches x 2 head-groups (4 heads each).

V2.1 design (vs baseline):
  - The steady loop is paced by ScalarE exp (64 x ~1.11us); all PE work
    (scores, attn@V, projections) is scheduled to hide under it.
  - Projections hoisted: qproj(.,0)/kproj(0,0)/vproj 0-3 run pre-loop so
    the exp stream starts ~5us in; remaining projections ride the loop
    slack in 256-wide halves.
  - attn PSUMs ([65,512] per head) are evicted to SBUF promptly at unit
    end (norm_start) so the next unit's accumulation never stalls.
  - reciprocal input packed [97,256] (4 x 256 q-halves on partitions
    0/32/64/96) - DVE reciprocal time scales with free size: 2x faster
    than [33,512].
  - out-proj split per (mo, t2): 2 matmuls + copy + DMA per group; only
    the 4 t2=1 groups (minus 2 pre-run m0 chunks) remain in the tail.
  - V ones-columns pre-memset in SBUF; vproj matmuls are plain 256-wide
    (no ones matmul, no interleaved weight upload).
PSUM: scores 1x[128,2,2,512] (4 banks) + attn 2 + proj pool 2 = 8.
Host folds V-bias through softmax: bout' = bout + Wout @ bv.
"""

import sys
import os

for _p in ("/opt/trn_rl_repo", "/root/.axon_site/_ro/trn_rl_repo"):
    if os.path.isdir(_p) and _p not in sys.path:
        sys.path.append(_p)

import numpy as np
import ml_dtypes

import concourse.bass as bass
import concourse.mybir as mybir
import concourse.tile as tile
from concourse.bass_utils import run_bass_kernel_spmd

BF16 = mybir.dt.bfloat16
F32 = mybir.dt.float32
NP_BF16 = ml_dtypes.bfloat16

AF = mybir.ActivationFunctionType


def _split_multi_waits(nc: bass.Bass) -> None:
    """The walrus build here allows only one sync-wait per instruction.
    Tile attaches several; hoist the extras onto same-engine NOPs placed
    immediately before the instruction (same per-engine program order)."""
    uid = 0
    for f in nc.m.functions:
        for bb in f.blocks:
            insts = bb.instructions
            out = []
            changed = False
            for inst in insts:
                si = inst.sync_info
                if si is not None and si.on_wait is not None and len(si.on_wait) > 1:
                    waits = list(si.on_wait)
                    for w in waits[:-1]:
                        nop = mybir.InstNoOp(
                            name=f"splitwait-{uid}",
                            engine=inst.engine,
                            ins=[],
                            outs=[],
                            sync_info=mybir.SyncInfo(on_wait=[w], on_update=[]),
                        )
                        uid += 1
                        out.append(nop)
                    inst.sync_info = mybir.SyncInfo(
                        on_wait=[waits[-1]], on_update=list(si.on_update or [])
                    )
                    changed = True
                out.append(inst)
            if changed:
                bb.instructions = out


def _build_program() -> bass.Bass:
    nc = bass.Bass()

    # all inputs host-pre-shuffled to partition-major contiguous layouts so
    # each DMA is one contiguous segment per partition (128 descriptors;
    # chunked rearranges cost 4x the descriptor-issue time)
    qx_d = nc.declare_dram_parameter("qx", [128, 2, 4, 512], BF16, isOutput=False)
    kvx_d = nc.declare_dram_parameter("kvx", [128, 4, 4, 512], BF16, isOutput=False)
    wq_d = nc.declare_dram_parameter("wq", [128, 4, 256], BF16, isOutput=False)
    wk_d = nc.declare_dram_parameter("wk", [128, 4, 256], BF16, isOutput=False)
    wv_d = nc.declare_dram_parameter("wv", [128, 4, 256], BF16, isOutput=False)
    wo_d = nc.declare_dram_parameter("wo", [128, 2, 512], BF16, isOutput=False)
    bq_d = nc.declare_dram_parameter("bq", [128, 2], F32, isOutput=False)
    bk_d = nc.declare_dram_parameter("bk", [128, 2], F32, isOutput=False)
    out_d = nc.declare_dram_parameter("out", [512, 1024], F32, isOutput=True)

    from contextlib import ExitStack

    with tile.TileContext(nc) as tc, ExitStack() as ctx:
        sb = ctx.enter_context(tc.tile_pool(name="sb", bufs=1))
        esb = ctx.enter_context(tc.tile_pool(name="esb", bufs=4))
        small = ctx.enter_context(tc.tile_pool(name="small", bufs=2))
        sc_ps = ctx.enter_context(tc.tile_pool(name="scps", bufs=1, space="PSUM"))
        at_ps = ctx.enter_context(tc.tile_pool(name="atps", bufs=1, space="PSUM"))
        pj_ps = ctx.enter_context(tc.tile_pool(name="pjps", bufs=2, space="PSUM"))
        dpool = ctx.enter_context(tc.tile_pool(name="dram", bufs=2, space="DRAM"))

        # ---------------- SBUF tiles (persistent) ----------------
        # qx: [part, tok-half, chan-chunk, 512tok]; kvx: [part, tok-quarter,
        # chan-chunk, 512tok] - matches the DRAM layout exactly
        qx_s = sb.tile([128, 2, 4, 512], BF16, name="qx", tag="qx")
        kvx_s = sb.tile([128, 4, 4, 512], BF16, name="kvx", tag="kvx")
        wq_s = sb.tile([128, 4, 256], BF16, name="wq", tag="wq")
        wk_s = sb.tile([128, 4, 256], BF16, name="wk", tag="wk")
        wv_s = sb.tile([128, 4, 256], BF16, name="wv", tag="wv")
        wo_s = sb.tile([128, 2, 512], BF16, name="wo", tag="wo")
        bq_s = sb.tile([128, 2], F32, name="bq", tag="bq")
        bk_s = sb.tile([128, 2], F32, name="bk", tag="bk")
        qt_s = [sb.tile([128, 1024], BF16, name=f"qt{m}", tag=f"qt{m}") for m in range(2)]
        kt_s = [sb.tile([128, 2048], BF16, name=f"kt{m}", tag=f"kt{m}") for m in range(2)]
        v_s = sb.tile([128, 16, 260], BF16, name="v", tag="v")
        ot_s = [sb.tile([128, 1024], BF16, name=f"ot{m}", tag=f"ot{m}") for m in range(2)]

        # scores: pool of [128,2,512] tiles (2 banks each, double-buffered).
        # Must be SEPARATE tile objects per iteration: a single persistent
        # tile with rotating slot slices makes exp(g) serialize against
        # scores(g+1) (coarse cross-slot dependency) and the loop runs 3x
        # slower.
        sc_tiles = {}

        # v ones columns (col 65j+64 of each kv-tile row block) = 1.0
        for j in range(4):
            nc.vector.memset(v_s[:, :, 65 * j + 64:65 * j + 65], 1.0)

        # ---------------- DMAs in consumption order ----------------
        # every transfer contiguous on both sides (128 descriptors each)
        nc.sync.dma_start(out=wq_s[:], in_=wq_d[:])
        nc.sync.dma_start(out=qx_s[:, 0, :, :], in_=qx_d[:, 0, :, :])
        nc.sync.dma_start(out=wk_s[:], in_=wk_d[:])
        nc.sync.dma_start(out=kvx_s[:, 0, :, :], in_=kvx_d[:, 0, :, :])
        nc.sync.dma_start(out=bq_s[:], in_=bq_d[:])
        nc.sync.dma_start(out=bk_s[:], in_=bk_d[:])
        nc.sync.dma_start(out=wv_s[:], in_=wv_d[:])
        for u in range(1, 4):
            nc.sync.dma_start(out=kvx_s[:, u, :, :], in_=kvx_d[:, u, :, :])
        nc.sync.dma_start(out=qx_s[:, 1, :, :], in_=qx_d[:, 1, :, :])
        nc.sync.dma_start(out=wo_s[:], in_=wo_d[:])

        # ---------------- building blocks ----------------
        def qproj_half(m, t, h):
            base = t * 512 + h * 256
            ps = pj_ps.tile([128, 256], F32, name="pp", tag="pp")
            for k in range(4):
                nc.tensor.matmul(
                    ps,
                    lhsT=wq_s[:, k, m * 128:(m + 1) * 128],
                    rhs=qx_s[:, t, k, h * 256:h * 256 + 256],
                    start=(k == 0), stop=(k == 3),
                )
            nc.vector.tensor_scalar_add(
                out=qt_s[m][:, base:base + 256], in0=ps,
                scalar1=bq_s[:, m:m + 1],
            )

        def kproj_half(m, u, h):
            base = u * 512 + h * 256
            ps = pj_ps.tile([128, 256], F32, name="pp", tag="pp")
            for k in range(4):
                nc.tensor.matmul(
                    ps,
                    lhsT=wk_s[:, k, m * 128:(m + 1) * 128],
                    rhs=kvx_s[:, u, k, h * 256:h * 256 + 256],
                    start=(k == 0), stop=(k == 3),
                )
            nc.vector.tensor_scalar_add(
                out=kt_s[m][:, base:base + 256], in0=ps,
                scalar1=bk_s[:, m:m + 1],
            )

        def vproj_tile(tt):
            # v_s[:, tt, 65j:65j+64] = token-major V for head j
            ps = pj_ps.tile([128, 256], F32, name="pp", tag="pp")
            for k in range(4):
                nc.tensor.matmul(
                    ps,
                    lhsT=kvx_s[:, tt // 4, k, (tt % 4) * 128:(tt % 4) * 128 + 128],
                    rhs=wv_s[:, k, :],
                    start=(k == 0), stop=(k == 3),
                )
            src = ps.rearrange("p (j d) -> p j d", j=4)
            dst_row = v_s[:, tt, :]
            dst = bass.AP(
                tensor=dst_row.tensor, offset=dst_row.offset,
                ap=[list(dst_row.ap[0]), [65, 4], [1, 64]],
            )
            nc.vector.tensor_copy(out=dst, in_=src)

        # ---------------- attention ----------------
        # unit u: m = u%2 (head pair), t = u//2 (512-token half);
        # order (0,0),(1,0),(0,1),(1,1)
        def unit_mt(u):
            return u % 2, u // 2

        def emit_scores(g):
            u, i = g // 16, g % 16
            m, t = unit_mt(u)
            ksl = slice(i * 128, (i + 1) * 128)
            qsl = slice(t * 512, (t + 1) * 512)
            with tc.high_priority():
                sc = sc_ps.tile([128, 2, 512], F32, name="sc", tag="sc", bufs=2)
                nc.tensor.matmul(
                    sc[:, 0, :], lhsT=kt_s[m][0:64, ksl], rhs=qt_s[m][0:64, qsl],
                    start=True, stop=True, tile_position=(0, 0),
                )
                nc.tensor.matmul(
                    sc[:, 1, :], lhsT=kt_s[m][64:128, ksl], rhs=qt_s[m][64:128, qsl],
                    start=True, stop=True, tile_position=(64, 0),
                )
            sc_tiles[g] = sc

        oh_tiles = {}

        def attn_iter(g, e_t):
            u, i = g // 16, g % 16
            m, _ = unit_mt(u)
            if i == 0:
                ohA = at_ps.tile([65, 512], F32, name="ohA", tag="ohA")
                ohB = at_ps.tile([65, 512], F32, name="ohB", tag="ohB")
                oh_tiles[u] = (ohA, ohB)
            ohA, ohB = oh_tiles[u]
            jA, jB = 2 * m, 2 * m + 1
            nc.tensor.matmul(
                ohA, lhsT=v_s[:, i, 65 * jA:65 * jA + 65], rhs=e_t[:, 0, :],
                start=(i == 0), stop=(i == 15),
            )
            nc.tensor.matmul(
                ohB, lhsT=v_s[:, i, 65 * jB:65 * jB + 65], rhs=e_t[:, 1, :],
                start=(i == 0), stop=(i == 15),
            )

        norm_state = {}

        def norm_start(u, evict=True):
            # evict attn PSUMs to SBUF promptly (frees the 2 banks for the
            # next unit), pack denominators [97,256], reciprocal, stage to
            # DRAM for the partition-broadcast re-read. The last unit skips
            # the eviction (nobody reuses its banks) and reads PSUM direct.
            ohA, ohB = oh_tiles.pop(u)
            if evict:
                ocp = small.tile([65, 2, 512], F32, name="ocp", tag="ocp", bufs=2)
                nc.vector.tensor_copy(out=ocp[:, 0, :], in_=ohA)
                nc.vector.tensor_copy(out=ocp[:, 1, :], in_=ohB)
                srcs = (ocp[:, 0, :], ocp[:, 1, :])
            else:
                srcs = (ohA, ohB)
            dst = small.tile([97, 256], F32, name="dst", tag="dst", bufs=2)
            for h in range(2):
                for q2 in range(2):
                    nc.vector.tensor_copy(
                        out=dst[64 * h + 32 * q2:64 * h + 32 * q2 + 1, :],
                        in_=srcs[h][64:65, 256 * q2:256 * q2 + 256],
                    )
            rec = small.tile([97, 256], F32, name="rec", tag="rec", bufs=2)
            nc.vector.reciprocal(out=rec, in_=dst)
            sd = dpool.tile([4, 256], F32, name="sd", tag="sd")
            for r in range(4):
                nc.sync.dma_start(out=sd[r:r + 1, :], in_=rec[32 * r:32 * r + 1, :])
            norm_state[u] = (srcs, sd)

        def norm_finish(u):
            m, t = unit_mt(u)
            qsl = slice(t * 512, (t + 1) * 512)
            srcs, sd = norm_state.pop(u)
            for h, base in ((0, 0), (1, 64)):
                row_ap = sd[2 * h:2 * h + 1, :]
                bsrc = bass.AP(tensor=row_ap.tensor, offset=row_ap.offset,
                               ap=[[0, 64], [1, 512]])
                bcs = small.tile([64, 512], F32, name="bcs", tag="bcs", bufs=4)
                nc.sync.dma_start(out=bcs, in_=bsrc)
                nc.vector.tensor_mul(ot_s[m][base:base + 64, qsl],
                                     srcs[h][0:64, :], bcs)

        # out-proj group (mo, t2): [128,512] psum, 2 matmuls (m=0,1)
        op_ps = {}

        def outproj_chunk(mo, t2, m, engine="vector"):
            key = (mo, t2)
            if key not in op_ps:
                op_ps[key] = pj_ps.tile([128, 512], F32, name="pp", tag="pp")
            ps = op_ps[key]
            nc.tensor.matmul(
                ps,
                lhsT=wo_s[:, m, mo * 128:(mo + 1) * 128],
                rhs=ot_s[m][:, t2 * 512:(t2 + 1) * 512],
                start=(m == 0), stop=(m == 1),
            )
            if m == 1:
                fo = small.tile([128, 512], F32, name="fo", tag="fo", bufs=2)
                if engine == "vector":
                    nc.vector.tensor_copy(out=fo, in_=ps)
                else:
                    nc.scalar.activation(out=fo, in_=ps, func=AF.Copy)
                nc.sync.dma_start(
                    out=out_d[mo * 128:(mo + 1) * 128, t2 * 512:(t2 + 1) * 512],
                    in_=fo,
                )
                del op_ps[key]

        def outproj_group(mo, t2, engine="vector"):
            outproj_chunk(mo, t2, 0, engine)
            outproj_chunk(mo, t2, 1, engine)

        # ---------------- pre-loop ----------------
        # minimal critical path to scores(0)/exp(0), then the rest
        qproj_half(0, 0, 0)
        qproj_half(0, 0, 1)
        kproj_half(0, 0, 0)
        kproj_half(0, 0, 1)
        emit_scores(0)
        qproj_half(1, 0, 0)
        qproj_half(1, 0, 1)
        for tt in range(4):
            vproj_tile(tt)

        # ---------------- extras schedule (per iteration g) ----------------
        extras = {g: [] for g in range(64)}
        pre = {}
        # kproj(0,1) at 0,1; (0,2) at 2,3; (0,3) at 6,7 (needed by 4/8/12)
        for idx, (u_, g0) in enumerate(((1, 0), (2, 2), (3, 6))):
            for h_ in range(2):
                extras[g0 + h_].append(lambda u_=u_, h_=h_: kproj_half(0, u_, h_))
        # vproj tiles 4..15 just-in-time-2 at iters 2..13
        for tt in range(4, 16):
            extras[max(0, tt - 2)].append(lambda tt=tt: vproj_tile(tt))
        # kproj(1,u) at 14..21 (needed by 16/20/24/28)
        g = 14
        for u_ in range(4):
            for h_ in range(2):
                extras[g].append(lambda u_=u_, h_=h_: kproj_half(1, u_, h_))
                g += 1
        # qproj halves for t=1 at g22..25 (needed by 32/48)
        g = 22
        for m_ in range(2):
            for h_ in range(2):
                extras[g].append(lambda m_=m_, h_=h_: qproj_half(m_, 1, h_))
                g += 1
        # norms: unit u ends at g=16u+15
        for u in range(3):
            pre[16 * u + 16] = [lambda u=u: norm_start(u)]
            extras[16 * u + 17].append(lambda u=u: norm_finish(u))
        # outproj t2=0 (units 0,1 normed by ~g35)
        for idx, mo in enumerate(range(4)):
            extras[38 + 4 * idx].append(lambda mo=mo: outproj_group(mo, 0))

        # ---------------- main loop ----------------
        # Order matters for the counter-based semaphores: scores(g+1) must
        # precede attnV(g) in PE program order, else exp(g+1)'s wait
        # threshold transitively includes attnV(g) (which itself waits
        # exp(g)) and the two engines fully serialize.
        for g in range(64):
            if g + 1 < 64:
                emit_scores(g + 1)
            for fn in pre.get(g, ()):
                fn()
            sc = sc_tiles.pop(g)
            e_t = esb.tile([128, 2, 512], BF16, name="e", tag="e")
            nc.scalar.activation(out=e_t[:], in_=sc[:],
                                 func=AF.Exp, scale=0.125)
            for fn in extras.get(g, ()):
                fn()
            attn_iter(g, e_t)

        # ---------------- tail ----------------
        norm_start(3, evict=False)
        norm_finish(3)
        outproj_group(0, 1, engine="scalar")
        outproj_group(1, 1, engine="vector")
        outproj_group(2, 1, engine="scalar")
        outproj_group(3, 1, engine="vector")

    _split_multi_waits(nc)
    return nc


_PROGRAM = None


def _get_program() -> bass.Bass:
    global _PROGRAM
    if _PROGRAM is None:
        _PROGRAM = _build_program()
    return _PROGRAM


def _chunkT(w):
    # [512, 256] (in-chan, out-chan) -> [128, 4, 256] partition-major
    return np.ascontiguousarray(w.reshape(4, 128, 256).transpose(1, 0, 2))


def _prep_core_inputs(c, q, kv, Wqkv, bqkv, Wout):
    b, g = c // 2, c % 2
    cs = slice(256 * g, 256 * g + 256)
    # activations: [chan 512, tok] -> [part 128, tok-group, chan-chunk 4, 512]
    qx = q[b].reshape(4, 128, 2, 512).transpose(1, 2, 0, 3)
    kvx = kv[b].reshape(4, 128, 4, 512).transpose(1, 2, 0, 3)
    wo = Wout[:, cs].T.reshape(2, 128, 512).transpose(1, 0, 2)
    return {
        "qx": np.ascontiguousarray(qx).astype(NP_BF16),
        "kvx": np.ascontiguousarray(kvx).astype(NP_BF16),
        "wq": _chunkT(Wqkv[cs, :].T).astype(NP_BF16),
        "wk": _chunkT(Wqkv[512 + 256 * g:512 + 256 * g + 256, :].T).astype(NP_BF16),
        "wv": _chunkT(Wqkv[1024 + 256 * g:1024 + 256 * g + 256, :].T).astype(NP_BF16),
        "wo": np.ascontiguousarray(wo).astype(NP_BF16),
        "bq": np.ascontiguousarray(bqkv[cs].reshape(2, 128).T).astype(np.float32),
        "bk": np.ascontiguousarray(bqkv[512 + 256 * g:512 + 256 * g + 256].reshape(2, 128).T).astype(np.float32),
    }


def kernel(q, kv, Wqkv, bqkv, Wout, bout):
    q = np.asarray(q, np.float32)
    kv = np.asarray(kv, np.float32)
    Wqkv = np.asarray(Wqkv, np.float32)
    bqkv = np.asarray(bqkv, np.float32)
    Wout = np.asarray(Wout, np.float32)
    bout = np.asarray(bout, np.float32)

    nc = _get_program()
    in_maps = [_prep_core_inputs(c, q, kv, Wqkv, bqkv, Wout) for c in range(8)]
    res = run_bass_kernel_spmd(nc, in_maps, list(range(8))).results

    # V-bias folds through softmax (rows sum to 1): bout' = bout + Wout @ bv
    bout_adj = bout + Wout @ bqkv[1024:1536]
    out = np.empty((4, 512, 32, 32), np.float32)
    for b in range(4):
        o = res[2 * b]["out"] + res[2 * b + 1]["out"] + bout_adj[:, None]
        out[b] = o.reshape(512, 32, 32)
    return out
